# revision 26
# baseline (speedup 1.0000x reference)
"""Trainium2 Bass kernel for a dense transformer block (self-attn + cross-attn
+ MLP, returns (x_out, mean cross-attn probs)).

Sharding: 8 cores = 2 batch rows x 4 query-token slices of 512. Each core
computes full-row K/V (duplicated within a batch-row group) and its own
512-query slice of everything else. No collectives.

Layout strategy per core:
  - activations token-major for LayerNorm, PE-transposed to feature-major
    [128, 6, T] strips for matmuls (weights stay in natural [d_in, d_out]
    layout).
  - attention scores computed kv-major (scoresT [kv, q]) so softmax exp is a
    single ACT pass PSUM->SBUF and AV needs no transposes; softmax
    denominators come from a ones-augmented V column; o is normalized by a
    broadcast-reciprocal multiply after AV.
  - cross-attn probs are normalized in-place (bf16) so the mean-over-heads
    output `a` can be accumulated with identity matmuls in PSUM.
  - q processed in halves of 256 inside attention to bound SBUF residency.
"""

import numpy as np

DIM, H, S, B = 768, 12, 2048, 2
HD = DIM // H
SQ = S // 4  # 512 query tokens per core
Q2 = SQ // 2  # q half width inside attention
DM = 2 * DIM  # MLP hidden
EPS = 1e-5
KC = DIM // 128  # 6 feature chunks
MC = DM // 128  # 12 mlp hidden chunks
TKV = S // 128  # 16 kv token chunks
TQ = SQ // 128  # 4 q token chunks
NSTRIP = S // 1024  # strips of 1024 tokens for full-row processing
VW = H * (HD + 1)  # 780, ones-augmented V width
N_CORES = 8

_CACHE = {}


def _build():
    from contextlib import ExitStack

    import concourse.bass as bass
    import concourse.tile as tile
    from concourse import bacc, mybir
    from concourse.bass import ts

    f32 = mybir.dt.float32
    f32r = mybir.dt.float32r
    bf16 = mybir.dt.bfloat16
    Alu = mybir.AluOpType
    Act = mybir.ActivationFunctionType

    nc = bacc.Bacc("TRN2", target_bir_lowering=False, debug=False)

    def din(name, shape, dt):
        return nc.dram_tensor(name, shape, dt, kind="ExternalInput").ap()

    xq_d = din("xq", [SQ, DIM], f32)
    xr_d = din("xr", [S, DIM], f32)
    cr_d = din("cr", [S, DIM], f32)
    w_qs = din("w_qs", [DIM, DIM], bf16)
    w_ks = din("w_ks", [DIM, DIM], bf16)
    w_vs = din("w_vs", [DIM, VW], bf16)
    w_os = din("w_os", [DIM, DIM], bf16)
    w_qc = din("w_qc", [DIM, DIM], bf16)
    w_kc = din("w_kc", [DIM, DIM], bf16)
    w_vc = din("w_vc", [DIM, VW], bf16)
    w_oc = din("w_oc", [DIM, DIM], bf16)
    w_1 = din("w_1", [DIM, DM], bf16)
    w_2 = din("w_2", [DM, DIM], bf16)
    bc_qs = din("bc_qs", [DIM], f32)
    bc_ks = din("bc_ks", [DIM], f32)
    bc_qc = din("bc_qc", [DIM], f32)
    bc_kc = din("bc_kc", [DIM], f32)
    bc_1 = din("bc_1", [DM], f32)
    br_vs = din("br_vs", [1, VW], bf16)
    br_os = din("br_os", [1, DIM], bf16)
    br_vc = din("br_vc", [1, VW], bf16)
    br_oc = din("br_oc", [1, DIM], bf16)
    br_2 = din("br_2", [1, DIM], bf16)
    ident_r_d = din("ident_r", [128, 128], f32r)
    ident_b_d = din("ident_b", [128, 128], bf16)
    ones_d = din("ones_r", [1, 128], bf16)

    y_d = nc.dram_tensor("y", [SQ, DIM], f32, kind="ExternalOutput").ap()
    aT_d = nc.dram_tensor("aT", [S, SQ], f32, kind="ExternalOutput").ap()

    def bcast_ap(row_ap, nparts):
        """0-partition-stride broadcast view of a [1, N] sbuf row."""
        return bass.AP(tensor=row_ap.tensor, offset=row_ap.offset,
                       ap=[[0, nparts]] + row_ap.ap[1:])

    ctx = ExitStack()
    with tile.TileContext(nc) as tc, ctx:
        # ---------------- persistent pools ----------------
        consts = ctx.enter_context(tc.tile_pool(name="consts", bufs=1))
        ident_r = consts.tile([128, 128], f32r, tag="ident_r")
        nc.scalar.dma_start(ident_r, ident_r_d)
        ident_b = consts.tile([128, 128], bf16, tag="ident_b")
        nc.scalar.dma_start(ident_b, ident_b_d)
        ones_r = consts.tile([1, 128], bf16, tag="ones_r")
        nc.scalar.dma_start(ones_r, ones_d)
        eps_t = consts.tile([128, 1], f32, tag="eps")
        nc.vector.memset(eps_t, EPS)

        def load_bcol(d, name):
            t = consts.tile([128, d.shape[0] // 128], f32, tag=f"bc_{name}")
            nc.scalar.dma_start(t, d.rearrange("(ko ki) -> ki ko", ki=128))
            return t

        def load_brow(d, name):
            t = consts.tile([1, d.shape[1]], bf16, tag=f"br_{name}")
            nc.scalar.dma_start(t, d)
            return t

        bcol = {n: load_bcol(d, n) for n, d in [
            ("qs", bc_qs), ("ks", bc_ks), ("qc", bc_qc), ("kc", bc_kc),
            ("b1", bc_1)]}
        brow = {n: load_brow(d, n) for n, d in [
            ("vs", br_vs), ("os", br_os), ("vc", br_vc), ("oc", br_oc),
            ("b2", br_2)]}

        persist = ctx.enter_context(tc.tile_pool(name="persist", bufs=1))
        xq_sb = persist.tile([128, TQ, DIM], f32, tag="xq")

        lnt = ctx.enter_context(tc.tile_pool(name="lnt", bufs=3))
        lns = ctx.enter_context(tc.tile_pool(name="lns", bufs=4))
        dnst_pool = ctx.enter_context(tc.tile_pool(name="dnst", bufs=3))
        rbc_pool = ctx.enter_context(tc.tile_pool(name="rbc", bufs=2))
        dram_pool = ctx.enter_context(tc.tile_pool(name="dscr", bufs=2,
                                                   space="DRAM"))
        fm_q = ctx.enter_context(tc.tile_pool(name="fm_q", bufs=1))
        # attention-phase pools, closed before the MLP stage
        actx = ctx.enter_context(ExitStack())
        attn_sb = actx.enter_context(tc.tile_pool(name="attn_sb", bufs=1))
        o_pool = actx.enter_context(tc.tile_pool(name="o_pool", bufs=1))

        # ---------------- helpers ----------------
        def ln_tile(xt, name, out_dtype=bf16, lnp=None):
            """LayerNorm one token-major [128, DIM] tile, return normalized."""
            xs = xt.rearrange("p (a b) -> p a b", b=256)
            stats = lns.tile([128, 3, 6], f32, tag="ln_st")
            for g in range(3):
                nc.vector.bn_stats(out=stats[:, g, :], in_=xs[:, g, :])
            mv = lns.tile([128, 2], f32, tag="ln_mv")
            nc.vector.bn_aggr(out=mv, in_=stats)
            nc.scalar.activation(out=mv[:, 1:2], in_=mv[:, 1:2],
                                 func=Act.Sqrt, bias=eps_t, scale=1.0)
            nc.vector.reciprocal(out=mv[:, 1:2], in_=mv[:, 1:2])
            lt = lnt.tile([128, DIM], out_dtype, tag="ln_out")
            nc.vector.tensor_scalar(
                out=lt, in0=xt, scalar1=mv[:, 0:1], scalar2=mv[:, 1:2],
                op0=Alu.subtract, op1=Alu.mult)
            return lt

        def transpose_to_fm(lt, fm_dst, i, lnp):
            """[128 tok, DIM] normalized tile -> fm_dst[:, :, 128*i:...]"""
            for g in range(2):
                pst = lnp.tile([128, 3, 128], bf16, tag="tp")
                for j in range(3):
                    nc.tensor.transpose(
                        pst[:, j, :], lt[:, ts(3 * g + j, 128)], ident_b)
                if g == 0:
                    nc.vector.tensor_copy(
                        out=fm_dst[:, 3 * g:3 * g + 3, ts(i, 128)], in_=pst)
                else:
                    nc.scalar.copy(
                        out=fm_dst[:, 3 * g:3 * g + 3, ts(i, 128)], in_=pst)

        def proj_fm(act_fm, tok0, w_sb, bc, out_sb, ntok, nchunks, psp,
                    relu=False, nmove=512):
            """out_sb[:, j, tok0:tok0+ntok] = act_fm.T @ W + b (feature-major).
            act_fm is a strip [128, KC, ntok]."""
            for j in range(nchunks):
                for t in range(ntok // nmove):
                    ps = psp.tile([128, nmove], f32, tag="proj")
                    for k in range(KC):
                        nc.tensor.matmul(
                            ps, w_sb[:, k, ts(j, 128)], act_fm[:, k, ts(t, nmove)],
                            start=(k == 0), stop=(k == KC - 1))
                    dst = out_sb[:, j, tok0 + t * nmove:tok0 + (t + 1) * nmove]
                    fn = Act.Relu if relu else Act.Identity
                    nc.scalar.activation(out=dst, in_=ps, func=fn,
                                         bias=bc[:, j:j + 1], scale=1.0)

        def proj_tm(act_fm, kchunks, w_sb, br, outw, psp, consume, ntok,
                    tstart=0):
            """token-major out[tok, outw]; consume(t, psum_tile)."""
            segs = [(0, 512), (512, outw - 512)] if outw > 512 else [(0, outw)]
            for t in range(ntok // 128):
                ps = psp.tile([128, outw], f32, tag="ptm")
                for (o0, ow) in segs:
                    for k in range(kchunks):
                        nc.tensor.matmul(
                            ps[:, o0:o0 + ow], act_fm[:, k, ts(t, 128)],
                            w_sb[:, k, o0:o0 + ow], start=(k == 0), stop=False)
                    nc.tensor.matmul(ps[:, o0:o0 + ow], ones_r,
                                     br[:, o0:o0 + ow], start=False, stop=True)
                consume(tstart + t, ps)

        def load_w(d, nchunks, tag, pool):
            t = pool.tile([128, nchunks, d.shape[1]], bf16, tag=tag)
            nc.scalar.dma_start(t, d.rearrange("(ko ki) n -> ki ko n", ki=128))
            return t

        def row_pipeline(src_d, K_fm, V_sb, wk, wv, bck, brv, lnp, psp, vpsp,
                         fms):
            """Full-row LN -> fm strips -> K (feature-major) + V (token-major,
            ones-augmented) projections, strip by strip."""
            for s in range(NSTRIP):
                fm = fms.tile([128, KC, 1024], bf16, tag="strip")
                for i in range(8):
                    xt = lnt.tile([128, DIM], f32, tag="ln_in")
                    nc.sync.dma_start(xt, src_d[ts(8 * s + i, 128), :])
                    lt = ln_tile(xt, f"r{s}{i}")
                    transpose_to_fm(lt, fm, i, lnp)
                proj_fm(fm, 1024 * s, wk, bck, K_fm, 1024, KC, psp)

                def vout(t, ps):
                    nc.scalar.copy(out=V_sb[:, t, :], in_=ps)

                proj_tm(fm, KC, wv, brv, VW, vpsp, vout, 1024, tstart=8 * s)

        def q_ln_fm(src_cb, name, lnp):
            """512-token LN -> feature-major [128, KC, 512] (shared tag)."""
            fm = fm_q.tile([128, KC, SQ], bf16, tag="qfm")
            for i in range(TQ):
                xt = lnt.tile([128, DIM], f32, tag="ln_in")
                src_cb(i, xt)
                lt = ln_tile(xt, f"{name}{i}")
                transpose_to_fm(lt, fm, i, lnp)
            return fm

        def attention(K_fm, Q_fm, V_sb, o_sb, rb_full, probs_pool, sc_ps,
                      oav_ps, group_cb=None, group_size=12, qsplit=1,
                      head_cb=None):
            """Per q-piece, per head: scoresT -> exp -> AV. o_sb [128, KC, SQ]
            bf16 numerators; rb_full [128, KC, SQ] f32 gets the per-head
            reciprocal denominator broadcast into its partition rows.
            head_cb(qh, h, pr, rcg) fires right after a head's AV."""
            qw = SQ // qsplit
            ncpp = 1024 // qw  # kv chunks per score psum tile (2 banks)
            pair = qsplit == 1  # emit head-pair scores back-to-back so the
            # two 64-row-group matmuls run concurrently in the PE array
            for qh in range(qsplit):
                q0 = qw * qh
                group, gtiles = [], []
                pr_pend = {}
                for h in range(H):
                    hi, hp = h // 2, (h % 2) * 64
                    if pair and h % 2 == 0:
                        pr_a = probs_pool.tile([128, TKV, qw], bf16,
                                               tag="probs")
                        pr_b = probs_pool.tile([128, TKV, qw], bf16,
                                               tag="probs")
                        prs = [pr_a, pr_b]
                        for kp in range(TKV // ncpp):
                            sp_a = sc_ps.tile([128, ncpp, qw], f32, tag="sc")
                            sp_b = sc_ps.tile([128, ncpp, qw], f32, tag="sc")
                            spp = [sp_a, sp_b]
                            for s4 in range(ncpp):
                                for e in range(2):
                                    nc.tensor.matmul(
                                        spp[e][:, s4, :],
                                        K_fm[64 * e:64 * e + 64, hi,
                                             ts(ncpp * kp + s4, 128)],
                                        Q_fm[64 * e:64 * e + 64, hi,
                                             q0:q0 + qw],
                                        start=True, stop=True)
                            for e in range(2):
                                nc.scalar.activation(
                                    out=prs[e][:, ncpp * kp:ncpp * kp + ncpp, :],
                                    in_=spp[e], func=Act.Exp)
                        pr_pend = {h: prs[0], h + 1: prs[1]}
                    if pair:
                        pr = pr_pend[h]
                    else:
                        pr = probs_pool.tile([128, TKV, qw], bf16, tag="probs")
                        for kp in range(TKV // ncpp):
                            sps = sc_ps.tile([128, ncpp, qw], f32, tag="sc")
                            for s4 in range(ncpp):
                                nc.tensor.matmul(
                                    sps[:, s4, :],
                                    K_fm[hp:hp + 64, hi, ts(ncpp * kp + s4, 128)],
                                    Q_fm[hp:hp + 64, hi, q0:q0 + qw],
                                    start=True, stop=True)
                            nc.scalar.activation(
                                out=pr[:, ncpp * kp:ncpp * kp + ncpp, :], in_=sps,
                                func=Act.Exp)
                    ov = oav_ps.tile([65, qw], f32, tag="oav")
                    for kc in range(TKV):
                        nc.tensor.matmul(ov, V_sb[:, kc, ts(h, HD + 1)],
                                         pr[:, kc, :],
                                         start=(kc == 0), stop=(kc == TKV - 1))
                    nc.vector.tensor_copy(out=o_sb[hp:hp + 64, hi, q0:q0 + qw],
                                          in_=ov[0:64, :])
                    dnst = dnst_pool.tile([1, qw], f32, tag="dnst")
                    nc.vector.tensor_copy(out=dnst, in_=ov[64:65, :])
                    rcg = dnst_pool.tile([1, qw], f32, tag="rcg")
                    scr = dnst_pool.tile([1, qw], f32, tag="rscr")
                    nc.vector.reciprocal_approx_accurate(out=rcg, in_=dnst,
                                                         scratch=scr)
                    rcg_d = dram_pool.tile([1, qw], f32, tag="rcg_d")
                    nc.sync.dma_start(out=rcg_d, in_=rcg)
                    nc.gpsimd.dma_start(
                        out=rb_full[hp:hp + 64, hi, q0:q0 + qw],
                        in_=bcast_ap(rcg_d, 64))
                    if head_cb is not None:
                        head_cb(qh, h, pr, rcg)
                    group.append(h)
                    gtiles.append(pr)
                    if len(group) == group_size:
                        if group_cb is not None:
                            group_cb(qh, list(group), list(gtiles))
                        group, gtiles = [], []

        def divide_o(o_sb, rb_full):
            for j in range(KC):
                nc.vector.tensor_mul(out=o_sb[:, j, :], in0=o_sb[:, j, :],
                                     in1=rb_full[:, j, :])

        # ================= stage 1+2: self projections =================
        K_fm = attn_sb.tile([128, KC, S], bf16, tag="K")
        V_sb = attn_sb.tile([128, TKV, VW], bf16, tag="V")
        Q_fm = attn_sb.tile([128, KC, SQ], bf16, tag="Q")
        o_sb = o_pool.tile([128, KC, SQ], bf16, tag="o")
        rb_full = o_pool.tile([128, KC, SQ], f32, tag="rbf")

        with (
            tc.tile_pool(name="lnp1", bufs=2, space="PSUM") as lnp,
            tc.tile_pool(name="psp1", bufs=2, space="PSUM") as psp,
            tc.tile_pool(name="vpsp1", bufs=2, space="PSUM") as vpsp,
        ):
            with (
                tc.tile_pool(name="wkv_s", bufs=1) as wkv,
                tc.tile_pool(name="strip1", bufs=2) as fms,
            ):
                wk = load_w(w_ks, KC, "wk", wkv)
                wv = load_w(w_vs, KC, "wv", wkv)
                row_pipeline(xr_d, K_fm, V_sb, wk, wv, bcol["ks"], brow["vs"],
                             lnp, psp, vpsp, fms)

            with tc.tile_pool(name="wq_s", bufs=1) as wq_p:
                wq = load_w(w_qs, KC, "wq", wq_p)

                def xq_src(i, xt):
                    nc.sync.dma_start(xt, xq_d[ts(i, 128), :])

                ln1q_fm = q_ln_fm(xq_src, "ln1q", lnp)
                proj_fm(ln1q_fm, 0, wq, bcol["qs"], Q_fm, SQ, KC, psp)

        # ================= stage 3: self attention =================
        with (
            tc.tile_pool(name="probs_s", bufs=4) as probs_s,
            tc.tile_pool(name="sc_s", bufs=3, space="PSUM") as sc_ps,
            tc.tile_pool(name="oav_s", bufs=2, space="PSUM") as oav_ps,
        ):
            attention(K_fm, Q_fm, V_sb, o_sb, rb_full, probs_s, sc_ps,
                      oav_ps, qsplit=1)

        # ======== stage 4a: cross K/V pipeline (fills the divide gap) ====
        K_fm_c = attn_sb.tile([128, KC, S], bf16, tag="K")
        V_sb_c = attn_sb.tile([128, TKV, VW], bf16, tag="V")
        with (
            tc.tile_pool(name="lnp2", bufs=2, space="PSUM") as lnp,
            tc.tile_pool(name="psp2", bufs=2, space="PSUM") as psp,
            tc.tile_pool(name="vpsp3", bufs=2, space="PSUM") as vpsp,
            tc.tile_pool(name="wkv_c", bufs=1) as wkvc,
            tc.tile_pool(name="strip2", bufs=2) as fms,
        ):
            wkc = load_w(w_kc, KC, "wkc", wkvc)
            wvc = load_w(w_vc, KC, "wvc", wkvc)
            row_pipeline(cr_d, K_fm_c, V_sb_c, wkc, wvc, bcol["kc"],
                         brow["vc"], lnp, psp, vpsp, fms)

        divide_o(o_sb, rb_full)

        for t in range(TQ):
            nc.sync.dma_start(xq_sb[:, t, :], xq_d[ts(t, 128), :])
        with (
            tc.tile_pool(name="wo_p", bufs=1) as wo_p,
            tc.tile_pool(name="vpsp2", bufs=2, space="PSUM") as vpsp2,
        ):
            wo = load_w(w_os, KC, "wo", wo_p)

            def ores(t, ps):
                nc.vector.tensor_add(out=xq_sb[:, t, :], in0=xq_sb[:, t, :],
                                     in1=ps)

            proj_tm(o_sb, KC, wo, brow["os"], DIM, vpsp2, ores, SQ)

        # ================= stage 4b: ln2(x1) + cross Q projection ========
        with (
            tc.tile_pool(name="lnp4", bufs=2, space="PSUM") as lnp4,
            tc.tile_pool(name="psp4", bufs=2, space="PSUM") as psp4,
            tc.tile_pool(name="wq_c", bufs=1) as wqc_p,
        ):
            def x1_src(i, xt):
                nc.vector.tensor_copy(out=xt, in_=xq_sb[:, i, :])

            ln2q_fm = q_ln_fm(x1_src, "ln2q", lnp4)
            wqc = load_w(w_qc, KC, "wqc", wqc_p)
            Q_fm_c = attn_sb.tile([128, KC, SQ], bf16, tag="Q")
            proj_fm(ln2q_fm, 0, wqc, bcol["qc"], Q_fm_c, SQ, KC, psp4)

        # ================= stage 5: cross attention + mean probs ========
        o_sb_c = o_pool.tile([128, KC, SQ], bf16, tag="o")
        rb_full_c = o_pool.tile([128, KC, SQ], f32, tag="rbf")
        with (
            tc.tile_pool(name="probs_c", bufs=5) as probs_c,
            tc.tile_pool(name="sc_c", bufs=2, space="PSUM") as sc_ps,
            tc.tile_pool(name="oav_c", bufs=2, space="PSUM") as oav_ps,
            tc.tile_pool(name="a_ps", bufs=2, space="PSUM") as a_ps,
            tc.tile_pool(name="a_sb_p", bufs=1) as a_sb_p,
        ):
            gidx = [0]

            def cross_head(qh, h, pr, rcg):
                # normalize this head's probs in place: pr *= 1/(H*denom)
                q0 = Q2 * qh
                rcbg = dnst_pool.tile([1, Q2], bf16, tag="rcbg")
                nc.vector.tensor_scalar(
                    out=rcbg, in0=rcg, scalar1=1.0 / H, scalar2=None,
                    op0=Alu.mult)
                rcb_d = dram_pool.tile([1, Q2], bf16, tag="rcb_d")
                nc.sync.dma_start(out=rcb_d, in_=rcbg)
                rbb = rbc_pool.tile([128, Q2], bf16, tag="rbb")
                nc.gpsimd.dma_start(out=rbb, in_=bcast_ap(rcb_d, 128))
                rbb_b = bass.AP(tensor=rbb.tensor, offset=rbb.offset,
                                ap=[rbb.ap[0], [0, TKV], rbb.ap[1]])
                nc.vector.tensor_mul(out=pr[:, :, :], in0=pr[:, :, :],
                                     in1=rbb_b)

            def cross_group(qh, heads, tiles):
                q0 = Q2 * qh
                if gidx[0] % 3 == 0:
                    a_sb = a_sb_p.tile([128, TKV, Q2], f32, tag="a_sb")
                    gidx.append(a_sb)
                a_sb = gidx[-1]
                g = gidx[0] % 3
                for kp in range(TKV // 2):
                    aps = a_ps.tile([128, 2, Q2], f32, tag="aacc")
                    for i, pr in enumerate(tiles):
                        nc.tensor.matmul(aps, ident_b,
                                         pr[:, 2 * kp:2 * kp + 2, :],
                                         start=(i == 0),
                                         stop=(i == len(tiles) - 1))
                    dst = a_sb[:, 2 * kp:2 * kp + 2, :]
                    if g == 0:
                        nc.vector.tensor_copy(out=dst, in_=aps)
                    else:
                        nc.vector.tensor_add(out=dst, in0=dst, in1=aps)
                    if g == 2:
                        for kc in (2 * kp, 2 * kp + 1):
                            nc.sync.dma_start(
                                out=aT_d[ts(kc, 128), q0:q0 + Q2],
                                in_=a_sb[:, kc, :])
                gidx[0] += 1

            attention(K_fm_c, Q_fm_c, V_sb_c, o_sb_c, rb_full_c, probs_c,
                      sc_ps, oav_ps, group_cb=cross_group, group_size=4,
                      qsplit=2, head_cb=cross_head)
        divide_o(o_sb_c, rb_full_c)

        with (
            tc.tile_pool(name="woc_p", bufs=1) as woc_p,
            tc.tile_pool(name="vpsp4", bufs=2, space="PSUM") as vpsp4,
        ):
            woc = load_w(w_oc, KC, "woc", woc_p)

            def ores_c(t, ps):
                nc.vector.tensor_add(out=xq_sb[:, t, :], in0=xq_sb[:, t, :],
                                     in1=ps)

            proj_tm(o_sb_c, KC, woc, brow["oc"], DIM, vpsp4, ores_c, SQ)

        actx.close()  # release attention-phase SBUF before MLP weights

        # ================= stage 6: MLP =================
        with (
            tc.tile_pool(name="lnp3", bufs=2, space="PSUM") as lnp,
            tc.tile_pool(name="psp3", bufs=2, space="PSUM") as psp,
            tc.tile_pool(name="vpsp5", bufs=2, space="PSUM") as vpsp,
            tc.tile_pool(name="w12", bufs=1) as w12,
            tc.tile_pool(name="h1_p", bufs=1) as h1_p,
        ):
            def x2_src(i, xt):
                nc.vector.tensor_copy(out=xt, in_=xq_sb[:, i, :])

            ln3q_fm = q_ln_fm(x2_src, "ln3q", lnp)
            w1 = load_w(w_1, KC, "w1", w12)
            h1_fm = h1_p.tile([128, MC, SQ], bf16, tag="h1")
            proj_fm(ln3q_fm, 0, w1, bcol["b1"], h1_fm, SQ, MC, psp, relu=True)
            w2 = load_w(w_2, MC, "w2", w12)

            def yout(t, ps):
                nc.vector.tensor_add(out=xq_sb[:, t, :], in0=xq_sb[:, t, :],
                                     in1=ps)
                nc.sync.dma_start(out=y_d[ts(t, 128), :], in_=xq_sb[:, t, :])

            proj_tm(h1_fm, MC, w2, brow["b2"], DIM, vpsp, yout, SQ)

    nc.compile()
    return nc


def _prep_inputs(x, c, params):
    """Host-side weight folding. Returns per-core in_maps."""
    x = np.asarray(x, np.float32)
    c = np.asarray(c, np.float32)
    p = {k: (np.asarray(v, np.float32) if not isinstance(v, dict) else
             {k2: np.asarray(v2, np.float32) for k2, v2 in v.items()})
         for k, v in params.items()}

    def fold(g, b, w, bias):  # LN affine folded into following linear
        return g[:, None] * w, b @ w + bias

    sc = 1.0 / np.sqrt(HD)

    def attn_w(ap, gq, bq_ln, gkv, bkv_ln):
        wq, bq = fold(gq, bq_ln, ap["wq"], ap["bq"])
        wq, bq = wq * sc, bq * sc
        wk, bk = fold(gkv, bkv_ln, ap["wk"], ap["bk"])
        wv, bv = fold(gkv, bkv_ln, ap["wv"], ap["bv"])
        wva = np.zeros((DIM, VW), np.float32)
        bva = np.zeros((VW,), np.float32)
        for h in range(H):
            wva[:, h * (HD + 1):h * (HD + 1) + HD] = wv[:, h * HD:(h + 1) * HD]
            bva[h * (HD + 1):h * (HD + 1) + HD] = bv[h * HD:(h + 1) * HD]
            bva[h * (HD + 1) + HD] = 1.0
        return wq, bq, wk, bk, wva, bva, ap["wo"], ap["bo"]

    qs, bqs, ks, bks, vs, bvs, os_, bos = attn_w(
        p["self"], p["ln1_g"], p["ln1_b"], p["ln1_g"], p["ln1_b"])
    qc, bqc, kc_, bkc, vc, bvc, oc, boc = attn_w(
        p["cross"], p["ln2_g"], p["ln2_b"], p["lnc_g"], p["lnc_b"])
    w1, b1 = fold(p["ln3_g"], p["ln3_b"], p["mlp_w1"], p["mlp_b1"])
    w2, b2 = p["mlp_w2"], p["mlp_b2"]

    import ml_dtypes

    bf = ml_dtypes.bfloat16
    common = {
        "w_qs": qs.astype(bf), "w_ks": ks.astype(bf), "w_vs": vs.astype(bf),
        "w_os": os_.astype(bf), "w_qc": qc.astype(bf), "w_kc": kc_.astype(bf),
        "w_vc": vc.astype(bf), "w_oc": oc.astype(bf),
        "w_1": w1.astype(bf), "w_2": w2.astype(bf),
        "bc_qs": bqs, "bc_ks": bks, "bc_qc": bqc, "bc_kc": bkc, "bc_1": b1,
        "br_vs": bvs[None, :].astype(bf), "br_os": bos[None, :].astype(bf),
        "br_vc": bvc[None, :].astype(bf), "br_oc": boc[None, :].astype(bf),
        "br_2": b2[None, :].astype(bf),
        "ident_r": np.eye(128, dtype=np.float32),
        "ident_b": np.eye(128, dtype=np.float32).astype(bf),
        "ones_r": np.ones((1, 128), np.float32).astype(bf),
    }
    common = {k: np.ascontiguousarray(v) for k, v in common.items()}
    in_maps = []
    for core in range(N_CORES):
        b, j = divmod(core, 4)
        m = dict(common)
        m["xq"] = np.ascontiguousarray(x[b, j * SQ:(j + 1) * SQ])
        m["xr"] = np.ascontiguousarray(x[b])
        m["cr"] = np.ascontiguousarray(c[b])
        in_maps.append(m)
    return in_maps


def get_nc():
    if "nc" not in _CACHE:
        _CACHE["nc"] = _build()
    return _CACHE["nc"]


def run(x, c, params, trace=False):
    from concourse.bass_utils import run_bass_kernel_spmd

    nc = get_nc()
    in_maps = _prep_inputs(x, c, params)
    res = run_bass_kernel_spmd(nc, in_maps, core_ids=list(range(N_CORES)),
                               trace=trace)
    x_out = np.empty((B, S, DIM), np.float32)
    a_out = np.empty((B, S, S), np.float32)
    for core in range(N_CORES):
        b, j = divmod(core, 4)
        x_out[b, j * SQ:(j + 1) * SQ] = res.results[core]["y"]
        a_out[b, j * SQ:(j + 1) * SQ, :] = res.results[core]["aT"].T
    return (x_out, a_out), res


def kernel(x, c, params):
    out, _ = run(x, c, params)
    return out


# revision 27
# speedup vs baseline: 1.0416x; 1.0416x over previous
"""Trainium2 Bass kernel for a dense transformer block (self-attn + cross-attn
+ MLP, returns (x_out, mean cross-attn probs)).

Sharding: 8 cores = 2 batch rows x 4 query-token slices of 512. Each core
computes full-row K/V (duplicated within a batch-row group) and its own
512-query slice of everything else. No collectives.

Layout strategy per core:
  - activations token-major for LayerNorm, PE-transposed to feature-major
    [128, 6, T] strips for matmuls (weights stay in natural [d_in, d_out]
    layout).
  - attention scores computed kv-major (scoresT [kv, q]) so softmax exp is a
    single ACT pass PSUM->SBUF and AV needs no transposes; softmax
    denominators come from a ones-augmented V column; o is normalized by a
    broadcast-reciprocal multiply after AV.
  - cross-attn probs are normalized in-place (bf16) so the mean-over-heads
    output `a` can be accumulated with identity matmuls in PSUM.
  - q processed in halves of 256 inside attention to bound SBUF residency.
"""

import numpy as np

DIM, H, S, B = 768, 12, 2048, 2
HD = DIM // H
SQ = S // 4  # 512 query tokens per core
Q2 = SQ // 2  # q half width inside attention
DM = 2 * DIM  # MLP hidden
EPS = 1e-5
KC = DIM // 128  # 6 feature chunks
MC = DM // 128  # 12 mlp hidden chunks
TKV = S // 128  # 16 kv token chunks
TQ = SQ // 128  # 4 q token chunks
NSTRIP = S // 1024  # strips of 1024 tokens for full-row processing
VW = H * (HD + 1)  # 780, ones-augmented V width
N_CORES = 8

_CACHE = {}


def _build():
    from contextlib import ExitStack

    import concourse.bass as bass
    import concourse.tile as tile
    from concourse import bacc, mybir
    from concourse.bass import ts

    f32 = mybir.dt.float32
    f32r = mybir.dt.float32r
    bf16 = mybir.dt.bfloat16
    Alu = mybir.AluOpType
    Act = mybir.ActivationFunctionType

    nc = bacc.Bacc("TRN2", target_bir_lowering=False, debug=False)

    def din(name, shape, dt):
        return nc.dram_tensor(name, shape, dt, kind="ExternalInput").ap()

    xq_d = din("xq", [SQ, DIM], f32)
    xr_d = din("xr", [S, DIM], f32)
    cr_d = din("cr", [S, DIM], f32)
    w_qs = din("w_qs", [DIM, DIM], bf16)
    w_ks = din("w_ks", [DIM, DIM], bf16)
    w_vs = din("w_vs", [DIM, VW], bf16)
    w_os = din("w_os", [DIM, DIM], bf16)
    w_qc = din("w_qc", [DIM, DIM], bf16)
    w_kc = din("w_kc", [DIM, DIM], bf16)
    w_vc = din("w_vc", [DIM, VW], bf16)
    w_oc = din("w_oc", [DIM, DIM], bf16)
    w_1 = din("w_1", [DIM, DM], bf16)
    w_2 = din("w_2", [DM, DIM], bf16)
    bc_qs = din("bc_qs", [DIM], f32)
    bc_ks = din("bc_ks", [DIM], f32)
    bc_qc = din("bc_qc", [DIM], f32)
    bc_kc = din("bc_kc", [DIM], f32)
    bc_1 = din("bc_1", [DM], f32)
    br_vs = din("br_vs", [1, VW], bf16)
    br_os = din("br_os", [1, DIM], bf16)
    br_vc = din("br_vc", [1, VW], bf16)
    br_oc = din("br_oc", [1, DIM], bf16)
    br_2 = din("br_2", [1, DIM], bf16)
    ident_r_d = din("ident_r", [128, 128], f32r)
    ident_b_d = din("ident_b", [128, 128], bf16)
    ones_d = din("ones_r", [1, 128], bf16)

    y_d = nc.dram_tensor("y", [SQ, DIM], f32, kind="ExternalOutput").ap()
    aT_d = nc.dram_tensor("aT", [S, SQ], f32, kind="ExternalOutput").ap()

    def bcast_ap(row_ap, nparts):
        """0-partition-stride broadcast view of a [1, N] sbuf row."""
        return bass.AP(tensor=row_ap.tensor, offset=row_ap.offset,
                       ap=[[0, nparts]] + row_ap.ap[1:])

    ctx = ExitStack()
    with tile.TileContext(nc) as tc, ctx:
        # ---------------- persistent pools ----------------
        consts = ctx.enter_context(tc.tile_pool(name="consts", bufs=1))
        ident_r = consts.tile([128, 128], f32r, tag="ident_r")
        nc.scalar.dma_start(ident_r, ident_r_d)
        ident_b = consts.tile([128, 128], bf16, tag="ident_b")
        nc.scalar.dma_start(ident_b, ident_b_d)
        ones_r = consts.tile([1, 128], bf16, tag="ones_r")
        nc.scalar.dma_start(ones_r, ones_d)
        eps_t = consts.tile([128, 1], f32, tag="eps")
        nc.vector.memset(eps_t, EPS)

        def load_bcol(d, name):
            t = consts.tile([128, d.shape[0] // 128], f32, tag=f"bc_{name}")
            nc.scalar.dma_start(t, d.rearrange("(ko ki) -> ki ko", ki=128))
            return t

        def load_brow(d, name):
            t = consts.tile([1, d.shape[1]], bf16, tag=f"br_{name}")
            nc.scalar.dma_start(t, d)
            return t

        bcol = {n: load_bcol(d, n) for n, d in [
            ("qs", bc_qs), ("ks", bc_ks), ("qc", bc_qc), ("kc", bc_kc),
            ("b1", bc_1)]}
        brow = {n: load_brow(d, n) for n, d in [
            ("vs", br_vs), ("os", br_os), ("vc", br_vc), ("oc", br_oc),
            ("b2", br_2)]}

        persist = ctx.enter_context(tc.tile_pool(name="persist", bufs=1))
        xq_sb = persist.tile([128, TQ, DIM], f32, tag="xq")

        lnt = ctx.enter_context(tc.tile_pool(name="lnt", bufs=3))
        lns = ctx.enter_context(tc.tile_pool(name="lns", bufs=4))
        dnst_pool = ctx.enter_context(tc.tile_pool(name="dnst", bufs=3))
        rbc_pool = ctx.enter_context(tc.tile_pool(name="rbc", bufs=2))
        dram_pool = ctx.enter_context(tc.tile_pool(name="dscr", bufs=2,
                                                   space="DRAM"))
        fm_q = ctx.enter_context(tc.tile_pool(name="fm_q", bufs=1))
        # attention-phase pools, closed before the MLP stage
        actx = ctx.enter_context(ExitStack())
        attn_sb = actx.enter_context(tc.tile_pool(name="attn_sb", bufs=1))
        o_pool = actx.enter_context(tc.tile_pool(name="o_pool", bufs=1))

        # ---------------- helpers ----------------
        def ln_tile(xt, name, out_dtype=bf16, lnp=None):
            """LayerNorm one token-major [128, DIM] tile, return normalized."""
            xs = xt.rearrange("p (a b) -> p a b", b=256)
            stats = lns.tile([128, 3, 6], f32, tag="ln_st")
            for g in range(3):
                nc.vector.bn_stats(out=stats[:, g, :], in_=xs[:, g, :])
            mv = lns.tile([128, 2], f32, tag="ln_mv")
            nc.vector.bn_aggr(out=mv, in_=stats)
            nc.scalar.activation(out=mv[:, 1:2], in_=mv[:, 1:2],
                                 func=Act.Sqrt, bias=eps_t, scale=1.0)
            nc.vector.reciprocal(out=mv[:, 1:2], in_=mv[:, 1:2])
            lt = lnt.tile([128, DIM], out_dtype, tag="ln_out")
            nc.vector.tensor_scalar(
                out=lt, in0=xt, scalar1=mv[:, 0:1], scalar2=mv[:, 1:2],
                op0=Alu.subtract, op1=Alu.mult)
            return lt

        def transpose_to_fm(lt, fm_dst, i, lnp):
            """[128 tok, DIM] normalized tile -> fm_dst[:, :, 128*i:...]"""
            for g in range(2):
                pst = lnp.tile([128, 3, 128], bf16, tag="tp")
                for j in range(3):
                    nc.tensor.transpose(
                        pst[:, j, :], lt[:, ts(3 * g + j, 128)], ident_b)
                if g == 0:
                    nc.vector.tensor_copy(
                        out=fm_dst[:, 3 * g:3 * g + 3, ts(i, 128)], in_=pst)
                else:
                    nc.scalar.copy(
                        out=fm_dst[:, 3 * g:3 * g + 3, ts(i, 128)], in_=pst)

        def proj_fm(act_fm, tok0, w_sb, bc, out_sb, ntok, nchunks, psp,
                    relu=False, nmove=512):
            """out_sb[:, j, tok0:tok0+ntok] = act_fm.T @ W + b (feature-major).
            act_fm is a strip [128, KC, ntok]."""
            for j in range(nchunks):
                for t in range(ntok // nmove):
                    ps = psp.tile([128, nmove], f32, tag="proj")
                    for k in range(KC):
                        nc.tensor.matmul(
                            ps, w_sb[:, k, ts(j, 128)], act_fm[:, k, ts(t, nmove)],
                            start=(k == 0), stop=(k == KC - 1))
                    dst = out_sb[:, j, tok0 + t * nmove:tok0 + (t + 1) * nmove]
                    fn = Act.Relu if relu else Act.Identity
                    nc.scalar.activation(out=dst, in_=ps, func=fn,
                                         bias=bc[:, j:j + 1], scale=1.0)

        def proj_tm(act_fm, kchunks, w_sb, br, outw, psp, consume, ntok,
                    tstart=0):
            """token-major out[tok, outw]; consume(t, psum_tile)."""
            segs = [(0, 512), (512, outw - 512)] if outw > 512 else [(0, outw)]
            for t in range(ntok // 128):
                ps = psp.tile([128, outw], f32, tag="ptm")
                for (o0, ow) in segs:
                    for k in range(kchunks):
                        nc.tensor.matmul(
                            ps[:, o0:o0 + ow], act_fm[:, k, ts(t, 128)],
                            w_sb[:, k, o0:o0 + ow], start=(k == 0), stop=False)
                    nc.tensor.matmul(ps[:, o0:o0 + ow], ones_r,
                                     br[:, o0:o0 + ow], start=False, stop=True)
                consume(tstart + t, ps)

        def load_w(d, nchunks, tag, pool):
            t = pool.tile([128, nchunks, d.shape[1]], bf16, tag=tag)
            nc.scalar.dma_start(t, d.rearrange("(ko ki) n -> ki ko n", ki=128))
            return t

        def row_pipeline(src_d, K_fm, V_sb, wk, wv, bck, brv, lnp, psp, vpsp,
                         fms):
            """Full-row LN -> fm strips -> K (feature-major) + V (token-major,
            ones-augmented) projections, strip by strip."""
            for s in range(NSTRIP):
                fm = fms.tile([128, KC, 1024], bf16, tag="strip")
                for i in range(8):
                    xt = lnt.tile([128, DIM], f32, tag="ln_in")
                    nc.sync.dma_start(xt, src_d[ts(8 * s + i, 128), :])
                    lt = ln_tile(xt, f"r{s}{i}")
                    transpose_to_fm(lt, fm, i, lnp)
                proj_fm(fm, 1024 * s, wk, bck, K_fm, 1024, KC, psp)

                def vout(t, ps):
                    nc.scalar.copy(out=V_sb[:, t, :], in_=ps)

                proj_tm(fm, KC, wv, brv, VW, vpsp, vout, 1024, tstart=8 * s)

        def q_ln_fm(src_cb, name, lnp):
            """512-token LN -> feature-major [128, KC, 512] (shared tag)."""
            fm = fm_q.tile([128, KC, SQ], bf16, tag="qfm")
            for i in range(TQ):
                xt = lnt.tile([128, DIM], f32, tag="ln_in")
                src_cb(i, xt)
                lt = ln_tile(xt, f"{name}{i}")
                transpose_to_fm(lt, fm, i, lnp)
            return fm

        def attention(K_fm, Q_fm, V_sb, o_sb, rb_full, probs_pool, sc_ps,
                      oav_ps, group_cb=None, group_size=12, qsplit=1,
                      head_cb=None):
            """Per q-piece, per head: scoresT -> exp -> AV. o_sb [128, KC, SQ]
            bf16 numerators; rb_full [128, KC, SQ] f32 gets the per-head
            reciprocal denominator broadcast into its partition rows.
            head_cb(qh, h, pr, rcg) fires right after a head's AV."""
            qw = SQ // qsplit
            ncpp = 1024 // qw  # kv chunks per score psum tile (2 banks)
            for qh in range(qsplit):
                q0 = qw * qh
                group, gtiles = [], []
                for h in range(H):
                    hi, hp = h // 2, (h % 2) * 64
                    pr = probs_pool.tile([128, TKV, qw], bf16, tag="probs")
                    for kp in range(TKV // ncpp):
                        sps = sc_ps.tile([128, ncpp, qw], f32, tag="sc")
                        for s4 in range(ncpp):
                            nc.tensor.matmul(
                                sps[:, s4, :],
                                K_fm[hp:hp + 64, hi, ts(ncpp * kp + s4, 128)],
                                Q_fm[hp:hp + 64, hi, q0:q0 + qw],
                                start=True, stop=True)
                        nc.scalar.activation(
                            out=pr[:, ncpp * kp:ncpp * kp + ncpp, :], in_=sps,
                            func=Act.Exp)
                    ov = oav_ps.tile([65, qw], f32, tag="oav")
                    for kc in range(TKV):
                        nc.tensor.matmul(ov, V_sb[:, kc, ts(h, HD + 1)],
                                         pr[:, kc, :],
                                         start=(kc == 0), stop=(kc == TKV - 1))
                    nc.vector.tensor_copy(out=o_sb[hp:hp + 64, hi, q0:q0 + qw],
                                          in_=ov[0:64, :])
                    dnst = dnst_pool.tile([1, qw], f32, tag="dnst")
                    nc.vector.tensor_copy(out=dnst, in_=ov[64:65, :])
                    rcg = dnst_pool.tile([1, qw], f32, tag="rcg")
                    scr = dnst_pool.tile([1, qw], f32, tag="rscr")
                    nc.vector.reciprocal_approx_accurate(out=rcg, in_=dnst,
                                                         scratch=scr)
                    rcg_d = dram_pool.tile([1, qw], f32, tag="rcg_d")
                    nc.sync.dma_start(out=rcg_d, in_=rcg)
                    nc.gpsimd.dma_start(
                        out=rb_full[hp:hp + 64, hi, q0:q0 + qw],
                        in_=bcast_ap(rcg_d, 64))
                    if head_cb is not None:
                        head_cb(qh, h, pr, rcg)
                    group.append(h)
                    gtiles.append(pr)
                    if len(group) == group_size:
                        if group_cb is not None:
                            group_cb(qh, list(group), list(gtiles))
                        group, gtiles = [], []

        def divide_o(o_sb, rb_full):
            for j in range(KC):
                nc.vector.tensor_mul(out=o_sb[:, j, :], in0=o_sb[:, j, :],
                                     in1=rb_full[:, j, :])

        # ================= stage 1+2: self projections =================
        K_fm = attn_sb.tile([128, KC, S], bf16, tag="K")
        V_sb = attn_sb.tile([128, TKV, VW], bf16, tag="V")
        Q_fm = attn_sb.tile([128, KC, SQ], bf16, tag="Q")
        o_sb = o_pool.tile([128, KC, SQ], bf16, tag="o")
        rb_full = o_pool.tile([128, KC, SQ], f32, tag="rbf")

        with (
            tc.tile_pool(name="lnp1", bufs=2, space="PSUM") as lnp,
            tc.tile_pool(name="psp1", bufs=2, space="PSUM") as psp,
            tc.tile_pool(name="vpsp1", bufs=2, space="PSUM") as vpsp,
        ):
            with (
                tc.tile_pool(name="wkv_s", bufs=1) as wkv,
                tc.tile_pool(name="strip1", bufs=2) as fms,
            ):
                wk = load_w(w_ks, KC, "wk", wkv)
                wv = load_w(w_vs, KC, "wv", wkv)
                row_pipeline(xr_d, K_fm, V_sb, wk, wv, bcol["ks"], brow["vs"],
                             lnp, psp, vpsp, fms)

            with tc.tile_pool(name="wq_s", bufs=1) as wq_p:
                wq = load_w(w_qs, KC, "wq", wq_p)

                def xq_src(i, xt):
                    nc.sync.dma_start(xt, xq_d[ts(i, 128), :])

                ln1q_fm = q_ln_fm(xq_src, "ln1q", lnp)
                proj_fm(ln1q_fm, 0, wq, bcol["qs"], Q_fm, SQ, KC, psp)

        # ================= stage 3: self attention =================
        with (
            tc.tile_pool(name="probs_s", bufs=3) as probs_s,
            tc.tile_pool(name="sc_s", bufs=3, space="PSUM") as sc_ps,
            tc.tile_pool(name="oav_s", bufs=2, space="PSUM") as oav_ps,
        ):
            attention(K_fm, Q_fm, V_sb, o_sb, rb_full, probs_s, sc_ps,
                      oav_ps, qsplit=1)

        # ======== stage 4a: cross K/V pipeline (fills the divide gap) ====
        K_fm_c = attn_sb.tile([128, KC, S], bf16, tag="K")
        V_sb_c = attn_sb.tile([128, TKV, VW], bf16, tag="V")
        with (
            tc.tile_pool(name="lnp2", bufs=2, space="PSUM") as lnp,
            tc.tile_pool(name="psp2", bufs=2, space="PSUM") as psp,
            tc.tile_pool(name="vpsp3", bufs=2, space="PSUM") as vpsp,
            tc.tile_pool(name="wkv_c", bufs=1) as wkvc,
            tc.tile_pool(name="strip2", bufs=2) as fms,
        ):
            wkc = load_w(w_kc, KC, "wkc", wkvc)
            wvc = load_w(w_vc, KC, "wvc", wkvc)
            row_pipeline(cr_d, K_fm_c, V_sb_c, wkc, wvc, bcol["kc"],
                         brow["vc"], lnp, psp, vpsp, fms)

        divide_o(o_sb, rb_full)

        for t in range(TQ):
            nc.sync.dma_start(xq_sb[:, t, :], xq_d[ts(t, 128), :])
        with (
            tc.tile_pool(name="wo_p", bufs=1) as wo_p,
            tc.tile_pool(name="vpsp2", bufs=2, space="PSUM") as vpsp2,
        ):
            wo = load_w(w_os, KC, "wo", wo_p)

            def ores(t, ps):
                nc.vector.tensor_add(out=xq_sb[:, t, :], in0=xq_sb[:, t, :],
                                     in1=ps)

            proj_tm(o_sb, KC, wo, brow["os"], DIM, vpsp2, ores, SQ)

        # ================= stage 4b: ln2(x1) + cross Q projection ========
        with (
            tc.tile_pool(name="lnp4", bufs=2, space="PSUM") as lnp4,
            tc.tile_pool(name="psp4", bufs=2, space="PSUM") as psp4,
            tc.tile_pool(name="wq_c", bufs=1) as wqc_p,
        ):
            def x1_src(i, xt):
                nc.vector.tensor_copy(out=xt, in_=xq_sb[:, i, :])

            ln2q_fm = q_ln_fm(x1_src, "ln2q", lnp4)
            wqc = load_w(w_qc, KC, "wqc", wqc_p)
            Q_fm_c = attn_sb.tile([128, KC, SQ], bf16, tag="Q")
            proj_fm(ln2q_fm, 0, wqc, bcol["qc"], Q_fm_c, SQ, KC, psp4)

        # ================= stage 5: cross attention + mean probs ========
        o_sb_c = o_pool.tile([128, KC, SQ], bf16, tag="o")
        rb_full_c = o_pool.tile([128, KC, SQ], f32, tag="rbf")
        with (
            tc.tile_pool(name="probs_c", bufs=5) as probs_c,
            tc.tile_pool(name="sc_c", bufs=2, space="PSUM") as sc_ps,
            tc.tile_pool(name="oav_c", bufs=2, space="PSUM") as oav_ps,
            tc.tile_pool(name="a_ps", bufs=2, space="PSUM") as a_ps,
            tc.tile_pool(name="a_sb_p", bufs=1) as a_sb_p,
        ):
            gidx = [0]

            def cross_head(qh, h, pr, rcg):
                # normalize this head's probs in place: pr *= 1/(H*denom)
                q0 = Q2 * qh
                rcbg = dnst_pool.tile([1, Q2], bf16, tag="rcbg")
                nc.vector.tensor_scalar(
                    out=rcbg, in0=rcg, scalar1=1.0 / H, scalar2=None,
                    op0=Alu.mult)
                rcb_d = dram_pool.tile([1, Q2], bf16, tag="rcb_d")
                nc.sync.dma_start(out=rcb_d, in_=rcbg)
                rbb = rbc_pool.tile([128, Q2], bf16, tag="rbb")
                nc.gpsimd.dma_start(out=rbb, in_=bcast_ap(rcb_d, 128))
                rbb_b = bass.AP(tensor=rbb.tensor, offset=rbb.offset,
                                ap=[rbb.ap[0], [0, TKV], rbb.ap[1]])
                nc.vector.tensor_mul(out=pr[:, :, :], in0=pr[:, :, :],
                                     in1=rbb_b)

            def cross_group(qh, heads, tiles):
                q0 = Q2 * qh
                if gidx[0] % 3 == 0:
                    a_sb = a_sb_p.tile([128, TKV, Q2], f32, tag="a_sb")
                    gidx.append(a_sb)
                a_sb = gidx[-1]
                g = gidx[0] % 3
                for kp in range(TKV // 2):
                    aps = a_ps.tile([128, 2, Q2], f32, tag="aacc")
                    for i, pr in enumerate(tiles):
                        nc.tensor.matmul(aps, ident_b,
                                         pr[:, 2 * kp:2 * kp + 2, :],
                                         start=(i == 0),
                                         stop=(i == len(tiles) - 1))
                    dst = a_sb[:, 2 * kp:2 * kp + 2, :]
                    if g == 0:
                        nc.vector.tensor_copy(out=dst, in_=aps)
                    else:
                        nc.vector.tensor_add(out=dst, in0=dst, in1=aps)
                    if g == 2:
                        for kc in (2 * kp, 2 * kp + 1):
                            nc.sync.dma_start(
                                out=aT_d[ts(kc, 128), q0:q0 + Q2],
                                in_=a_sb[:, kc, :])
                gidx[0] += 1

            attention(K_fm_c, Q_fm_c, V_sb_c, o_sb_c, rb_full_c, probs_c,
                      sc_ps, oav_ps, group_cb=cross_group, group_size=4,
                      qsplit=2, head_cb=cross_head)
        divide_o(o_sb_c, rb_full_c)

        with (
            tc.tile_pool(name="woc_p", bufs=1) as woc_p,
            tc.tile_pool(name="vpsp4", bufs=2, space="PSUM") as vpsp4,
        ):
            woc = load_w(w_oc, KC, "woc", woc_p)

            def ores_c(t, ps):
                nc.vector.tensor_add(out=xq_sb[:, t, :], in0=xq_sb[:, t, :],
                                     in1=ps)

            proj_tm(o_sb_c, KC, woc, brow["oc"], DIM, vpsp4, ores_c, SQ)

        actx.close()  # release attention-phase SBUF before MLP weights

        # ================= stage 6: MLP =================
        with (
            tc.tile_pool(name="lnp3", bufs=2, space="PSUM") as lnp,
            tc.tile_pool(name="psp3", bufs=2, space="PSUM") as psp,
            tc.tile_pool(name="vpsp5", bufs=2, space="PSUM") as vpsp,
            tc.tile_pool(name="w12", bufs=1) as w12,
            tc.tile_pool(name="h1_p", bufs=1) as h1_p,
        ):
            def x2_src(i, xt):
                nc.vector.tensor_copy(out=xt, in_=xq_sb[:, i, :])

            ln3q_fm = q_ln_fm(x2_src, "ln3q", lnp)
            w1 = load_w(w_1, KC, "w1", w12)
            h1_fm = h1_p.tile([128, MC, SQ], bf16, tag="h1")
            proj_fm(ln3q_fm, 0, w1, bcol["b1"], h1_fm, SQ, MC, psp, relu=True)
            w2 = load_w(w_2, MC, "w2", w12)

            def yout(t, ps):
                nc.vector.tensor_add(out=xq_sb[:, t, :], in0=xq_sb[:, t, :],
                                     in1=ps)
                nc.sync.dma_start(out=y_d[ts(t, 128), :], in_=xq_sb[:, t, :])

            proj_tm(h1_fm, MC, w2, brow["b2"], DIM, vpsp, yout, SQ)

    nc.compile()
    return nc


def _prep_inputs(x, c, params):
    """Host-side weight folding. Returns per-core in_maps."""
    x = np.asarray(x, np.float32)
    c = np.asarray(c, np.float32)
    p = {k: (np.asarray(v, np.float32) if not isinstance(v, dict) else
             {k2: np.asarray(v2, np.float32) for k2, v2 in v.items()})
         for k, v in params.items()}

    def fold(g, b, w, bias):  # LN affine folded into following linear
        return g[:, None] * w, b @ w + bias

    sc = 1.0 / np.sqrt(HD)

    def attn_w(ap, gq, bq_ln, gkv, bkv_ln):
        wq, bq = fold(gq, bq_ln, ap["wq"], ap["bq"])
        wq, bq = wq * sc, bq * sc
        wk, bk = fold(gkv, bkv_ln, ap["wk"], ap["bk"])
        wv, bv = fold(gkv, bkv_ln, ap["wv"], ap["bv"])
        wva = np.zeros((DIM, VW), np.float32)
        bva = np.zeros((VW,), np.float32)
        for h in range(H):
            wva[:, h * (HD + 1):h * (HD + 1) + HD] = wv[:, h * HD:(h + 1) * HD]
            bva[h * (HD + 1):h * (HD + 1) + HD] = bv[h * HD:(h + 1) * HD]
            bva[h * (HD + 1) + HD] = 1.0
        return wq, bq, wk, bk, wva, bva, ap["wo"], ap["bo"]

    qs, bqs, ks, bks, vs, bvs, os_, bos = attn_w(
        p["self"], p["ln1_g"], p["ln1_b"], p["ln1_g"], p["ln1_b"])
    qc, bqc, kc_, bkc, vc, bvc, oc, boc = attn_w(
        p["cross"], p["ln2_g"], p["ln2_b"], p["lnc_g"], p["lnc_b"])
    w1, b1 = fold(p["ln3_g"], p["ln3_b"], p["mlp_w1"], p["mlp_b1"])
    w2, b2 = p["mlp_w2"], p["mlp_b2"]

    import ml_dtypes

    bf = ml_dtypes.bfloat16
    common = {
        "w_qs": qs.astype(bf), "w_ks": ks.astype(bf), "w_vs": vs.astype(bf),
        "w_os": os_.astype(bf), "w_qc": qc.astype(bf), "w_kc": kc_.astype(bf),
        "w_vc": vc.astype(bf), "w_oc": oc.astype(bf),
        "w_1": w1.astype(bf), "w_2": w2.astype(bf),
        "bc_qs": bqs, "bc_ks": bks, "bc_qc": bqc, "bc_kc": bkc, "bc_1": b1,
        "br_vs": bvs[None, :].astype(bf), "br_os": bos[None, :].astype(bf),
        "br_vc": bvc[None, :].astype(bf), "br_oc": boc[None, :].astype(bf),
        "br_2": b2[None, :].astype(bf),
        "ident_r": np.eye(128, dtype=np.float32),
        "ident_b": np.eye(128, dtype=np.float32).astype(bf),
        "ones_r": np.ones((1, 128), np.float32).astype(bf),
    }
    common = {k: np.ascontiguousarray(v) for k, v in common.items()}
    in_maps = []
    for core in range(N_CORES):
        b, j = divmod(core, 4)
        m = dict(common)
        m["xq"] = np.ascontiguousarray(x[b, j * SQ:(j + 1) * SQ])
        m["xr"] = np.ascontiguousarray(x[b])
        m["cr"] = np.ascontiguousarray(c[b])
        in_maps.append(m)
    return in_maps


def get_nc():
    if "nc" not in _CACHE:
        _CACHE["nc"] = _build()
    return _CACHE["nc"]


def run(x, c, params, trace=False):
    from concourse.bass_utils import run_bass_kernel_spmd

    nc = get_nc()
    in_maps = _prep_inputs(x, c, params)
    res = run_bass_kernel_spmd(nc, in_maps, core_ids=list(range(N_CORES)),
                               trace=trace)
    x_out = np.empty((B, S, DIM), np.float32)
    a_out = np.empty((B, S, S), np.float32)
    for core in range(N_CORES):
        b, j = divmod(core, 4)
        x_out[b, j * SQ:(j + 1) * SQ] = res.results[core]["y"]
        a_out[b, j * SQ:(j + 1) * SQ, :] = res.results[core]["aT"].T
    return (x_out, a_out), res


def kernel(x, c, params):
    out, _ = run(x, c, params)
    return out
